# Initial kernel scaffold
#
"""ViT attention block (B=8, N=1024, dim=1024, heads=16, d_k=64) on 8 trn2 NeuronCores.

Sharding: data-parallel over batch (1 batch per core), weights replicated.
No collectives needed; each core computes its batch's full attention output.

Per-core algorithm (all matmuls on TensorE contract over the partition dim):
  - host pre-transposes x[b] -> xT [dim, tokens] so QKV projections can use
    w_qkv (natural layout) as the stationary operand.
  - QT/KT = (w_qkv[:, :2048]).T @ xT  -> [2048, tokens]; head pair 2t,2t+1
    lives in partition-tile t ([128, 1024]), i.e. heads' d_k=64 rows stacked.
  - V = xT.T @ w_qkv[:, 2048:]       -> [tokens, 1024], stored with a
    constant-1 column appended per head (65 cols/head) so the PV matmul
    produces softmax row-sums for free.
  - per head pair: S^T[m,n] = (KT tile).T @ QT (K=64 contraction; the two
    heads run as concurrent row-group matmuls via tile_position). S^T psum
    is chunked into two [128, 1024] tiles (2 banks each, double-buffered)
    so the exp of chunk i never blocks the S^T matmuls of chunk i+1 on a
    PSUM bank. exp(scale*S^T) on ScalarE streams per-chunk into ET bf16,
    laid out [128, (nh, h, 512)] so both exp chunks and PV rhs slices are
    contiguous. (max-subtraction is skipped: |scale*S| <~ 2 here, exp is
    exact-safe and softmax is shift-invariant.)
  - PV: out^T[d'+1, n] = V'.T @ E^T accumulated over m tiles; row 64 is the
    softmax denominator. Stage to SBUF, then: SBUF->SBUF reshape DMA of the
    denominator row to [128, 8] (reciprocal is ~12 cyc/elem on DVE, so it
    must run on the wide-partition layout), reciprocal, bounce through a
    DRAM row, and a stride-0-partition broadcast read back to [64, 1024].
    The normalize multiplies run on GpSimd (SBUF-only operands) to keep
    the DVE free for psum evictions.
  - final = attnT.T @ w_out + b_out, accumulated in three waves so no
    projection matmul ever waits on a late normalize chain: wave A
    (pairs 0-4, inside pair-7's loop, bias folded) -> oacc; wave B1
    (pairs 5-6, early drain, += oacc in place); wave B2 (pair 7, late
    drain) -> evict fp32 + out DMA, alternating sync/gpsimd DMA queues
    to double drain write bandwidth.

Startup: inputs land via 7 large DMA descriptors (the sync queue issues
descriptors at ~600 ns each, so many small DMAs would serialize the
start): xT in 2, pair-0 q/k column blocks via strided 3D APs, wv, wout,
bias. Loop fillers stream the remaining q/k blocks one pair ahead.
"""

import os
import numpy as np
import ml_dtypes

import concourse.bass as bass
from concourse import bacc
import concourse.mybir as mybir
import concourse.tile as tile
from concourse.bass_utils import run_bass_kernel_spmd

P = 128
N_TOK = 1024
DIM = 1024
HEADS = 16
D_K = 64
N_CORES = 8
SCALE = D_K ** -0.5  # 0.125

NP_T = N_TOK // P   # 8 token tiles
DP = DIM // P       # 8 dim tiles
NPAIRS = HEADS // 2  # 8 head pairs
VW = D_K + 1        # 65: V columns per head incl. ones column

# matmul operand dtype: "bf16" | "fp32r" | "fp32"
MM_DTYPE = os.environ.get("KERNEL_MM_DTYPE", "bf16")
_DT = {
    "bf16": mybir.dt.bfloat16,
    "fp32r": mybir.dt.float32r,
    "fp32": mybir.dt.float32,
}[MM_DTYPE]
_NPDT = {"bf16": ml_dtypes.bfloat16, "fp32r": np.float32, "fp32": np.float32}[MM_DTYPE]

F32 = mybir.dt.float32


def build_program():
    nc = bacc.Bacc("TRN2", target_bir_lowering=False, debug=False)

    xT = nc.dram_tensor("xT", [DIM, N_TOK], _DT, kind="ExternalInput").ap()
    wqkv = nc.dram_tensor("w_qkv", [DIM, 3 * DIM], _DT, kind="ExternalInput").ap()
    wout = nc.dram_tensor("w_out", [DIM, DIM], _DT, kind="ExternalInput").ap()
    bout = nc.dram_tensor("b_out", [DIM], F32, kind="ExternalInput").ap()
    out = nc.dram_tensor("out", [N_TOK, DIM], F32, kind="ExternalOutput").ap()
    # reciprocal'd softmax denominator bounce rows (one per head)
    rs_dram = nc.dram_tensor("rs_scratch", [HEADS, N_TOK], F32).ap()

    def wq_block(j):
        """One DMA-able view of w_qkv column block j (128 cols) across all
        1024 rows: [p=128, k=8, c=128] -> lands as an SBUF [128, 1024] tile
        whose k-th 128-col slice is w_qkv[k*128:(k+1)*128, j*128:(j+1)*128]."""
        return bass.AP(
            tensor=wqkv.tensor, offset=wqkv.offset + j * P,
            ap=[[3 * DIM, P], [P * 3 * DIM, DP], [1, P]],
        )

    with tile.TileContext(nc) as tc:
        with (
            tc.tile_pool(name="persist", bufs=1) as persist,
            tc.tile_pool(name="qkt", bufs=5) as qktp,
            tc.tile_pool(name="wqkb", bufs=2) as wqkb,
            tc.tile_pool(name="wvoa", bufs=1) as wvoa,
            tc.tile_pool(name="et", bufs=18) as etp,
            tc.tile_pool(name="stg", bufs=3) as stgp,
            tc.tile_pool(name="small", bufs=2) as small,
        ):
            v_sb = []      # per token-tile: [128, 16*65]
            attnT_sb = []  # per pair: [128, 1024] = two heads' [64, n]
            for j in range(NP_T):
                v_sb.append(persist.tile([P, HEADS * VW], _DT, tag=f"v{j}",
                                         name=f"v{j}"))
            for p in range(NPAIRS):
                attnT_sb.append(persist.tile([P, N_TOK], _DT, tag=f"attnT{p}",
                                             name=f"attnT{p}"))

            # ---- input DMAs spread across 4 queues: each queue issues
            # descriptors at ~600 ns, so parallel queues beat one long
            # chain. sync: xT (arrival-paced thirds); vector: pair-0 wq
            # per-k tiles; scalar: pair-0 wk per-k tiles; gpsimd: wv,
            # wout, bias. ----
            xT_all = persist.tile([P, DP * N_TOK], _DT, tag="xT", name="xT")
            for k in range(DP):
                nc.sync.dma_start(
                    xT_all[:, k * N_TOK:(k + 1) * N_TOK],
                    xT[k * P:(k + 1) * P, :])

            def xs(k):
                return xT_all[:, k * N_TOK:(k + 1) * N_TOK]

            wq_cur = wqkb.tile([P, DIM], _DT, tag="wq", name="wq0")
            wk_cur = wqkb.tile([P, DIM], _DT, tag="wk", name="wk0")
            for k in range(DP):
                nc.scalar.dma_start(wq_cur[:, k * P:(k + 1) * P],
                                    wqkv[k * P:(k + 1) * P, 0:P])
                nc.sync.dma_start(wk_cur[:, k * P:(k + 1) * P],
                                  wqkv[k * P:(k + 1) * P,
                                       DP * P:DP * P + P])

            wv_all = wvoa.tile([P, DP * DIM], _DT, tag="wvoa", name="wv")
            for k0 in range(0, DP, 2):
                src = bass.AP(
                    tensor=wqkv.tensor,
                    offset=wqkv.offset + k0 * P * 3 * DIM + 2 * DIM,
                    ap=[[3 * DIM, P], [P * 3 * DIM, 2], [1, DIM]])
                nc.gpsimd.dma_start(
                    wv_all[:, k0 * DIM:(k0 + 2) * DIM], src)

            def wvs(k):
                return wv_all[:, k * DIM:(k + 1) * DIM]

            wout_all = persist.tile([P, DP * DIM], _DT, tag="wout",
                                    name="wout")
            for k0 in range(0, DP, 2):
                src = bass.AP(
                    tensor=wout.tensor, offset=wout.offset + k0 * P * DIM,
                    ap=[[DIM, P], [P * DIM, 2], [1, DIM]])
                nc.gpsimd.dma_start(
                    wout_all[:, k0 * DIM:(k0 + 2) * DIM], src)

            def wouts(k):
                return wout_all[:, k * DIM:(k + 1) * DIM]

            bias_bc = persist.tile([P, DIM], F32, tag="bias")
            bias_in = bass.AP(tensor=bout.tensor, offset=bout.offset,
                              ap=[[0, P]] + list(bout.ap))
            nc.gpsimd.dma_start(bias_bc[:], bias_in)
            for j in range(NP_T):
                nc.vector.memset(
                    v_sb[j][:].rearrange("p (h x) -> p h x", x=VW)[:, :, D_K:],
                    1.0)

            def emit_qkt(j, wblk, pool):
                """QKT M-tile j ([128, tokens] slice of QKV^T) in full."""
                ps = pool.tile([P, N_TOK], F32, tag="pq", name=f"psqk{j}")
                for k in range(DP):
                    for nh in range(2):
                        nc.tensor.matmul(
                            ps[:, nh * 512:(nh + 1) * 512],
                            lhsT=wblk[:, k * P:(k + 1) * P],
                            rhs=xs(k)[:, nh * 512:(nh + 1) * 512],
                            start=(k == 0), stop=(k == DP - 1),
                        )
                t = qktp.tile([P, N_TOK], _DT, tag="qkt", name=f"qkt{j}")
                nc.vector.tensor_copy(out=t[:], in_=ps[:])
                return t

            # ============ phase 1: pair-0 QT/KT, k-interleaved so both
            # psums accumulate as the xT / weight DMAs arrive ============
            with tc.tile_pool(name="pq1", bufs=2, space="PSUM") as pq1:
                psq = pq1.tile([P, N_TOK], F32, tag="pq", name="psqk0")
                psk = pq1.tile([P, N_TOK], F32, tag="pq", name="psqk8")
                for k in range(DP):
                    for ps, wblk in ((psq, wq_cur), (psk, wk_cur)):
                        for nh in range(2):
                            nc.tensor.matmul(
                                ps[:, nh * 512:(nh + 1) * 512],
                                lhsT=wblk[:, k * P:(k + 1) * P],
                                rhs=xs(k)[:, nh * 512:(nh + 1) * 512],
                                start=(k == 0), stop=(k == DP - 1),
                            )
                qt_cur = qktp.tile([P, N_TOK], _DT, tag="qkt", name="qkt0")
                nc.vector.tensor_copy(out=qt_cur[:], in_=psq[:])
                kt_cur = qktp.tile([P, N_TOK], _DT, tag="qkt", name="qkt8")
                nc.vector.tensor_copy(out=kt_cur[:], in_=psk[:])

            # ============ phase 2: pipelined attention ============
            with (
                tc.tile_pool(name="pst", bufs=2, space="PSUM") as pstp,
                tc.tile_pool(name="pq2", bufs=1, space="PSUM") as pq2,
                tc.tile_pool(name="pgen", bufs=2, space="PSUM") as pgen,
            ):
                et_tiles = {}   # (pair, mt) -> ET tile [128, 2048]
                inflight = {}   # accumulation state for pv / filler psums

                def emit_v_chunk(j, nh):
                    """8 matmuls: V[j-tile, nh-half] = xT.T @ wv, evicted
                    into v_sb[j] (heads nh*8..nh*8+7, 64 cols each)."""
                    ps = pgen.tile([P, 512], F32, tag="pv",
                                   name=f"psv{j}_{nh}")
                    for k in range(DP):
                        nc.tensor.matmul(
                            ps[:],
                            lhsT=xs(k)[:, j * P:(j + 1) * P],
                            rhs=wvs(k)[:, nh * 512:(nh + 1) * 512],
                            start=(k == 0), stop=(k == DP - 1),
                        )
                    nc.vector.tensor_copy(
                        out=v_sb[j][:].rearrange(
                            "p (h x) -> p h x", x=VW)[:, 8 * nh:8 * nh + 8,
                                                      :D_K],
                        in_=ps[:].rearrange("p (h d) -> p h d", d=D_K),
                    )

                def normalize_evict(p, h, stg):
                    """Normalize the staged PV result by the softmax
                    denominator (row 64) and write into attnT_sb[p].
                    Latency chains ride the gpsimd SWDGE queue to keep the
                    sync queue free for weight streams; the last pair uses
                    the (by then idle) sync queue for lower latency."""
                    hg = 2 * p + h
                    dma = nc.gpsimd.dma_start if p < NPAIRS - 1 else \
                        nc.sync.dma_start
                    rsp = small.tile([P, NP_T], F32, tag="rsp",
                                     name=f"rsp{hg}")
                    dma(rsp[:], stg[D_K:VW, :].rearrange(
                        "o (p i) -> o p i", p=P))
                    rsq = small.tile([P, NP_T], F32, tag="rsq",
                                     name=f"rsq{hg}")
                    nc.vector.reciprocal(rsq[:], rsp[:])
                    dma(rs_dram[hg].rearrange("(p i) -> p i", p=P), rsq[:])
                    rs_row = rs_dram[hg:hg + 1, :]
                    rs_bc = bass.AP(tensor=rs_row.tensor, offset=rs_row.offset,
                                    ap=[[0, D_K], list(rs_row.ap)[-1]])
                    rrec = small.tile([D_K, N_TOK], F32, tag="rrec",
                                      name=f"rrec{hg}")
                    dma(rrec[:], rs_bc)
                    if h == 0:
                        nc.vector.tensor_mul(out=attnT_sb[p][0:D_K, :],
                                             in0=stg[0:D_K, :], in1=rrec[:])
                    else:
                        tmp = small.tile([D_K, N_TOK], _DT, tag="oddtmp",
                                         name=f"oddtmp{hg}")
                        nc.vector.tensor_mul(out=tmp[:],
                                             in0=stg[0:D_K, :], in1=rrec[:])
                        dma(attnT_sb[p][D_K:P, :], tmp[:])

                def pv_chunk(p, slot8):
                    """4 PV matmuls for pair p. Passes of 8 MMs: (h, nh) =
                    slot8//2, each pass covers all m-tiles in 2 slots using
                    a 1-bank psum tile; evicted into the stg half."""
                    h, nh = slot8 // 4, (slot8 // 2) % 2
                    hg = 2 * p + h
                    half = slot8 % 2  # first or second 4 m-tiles
                    if half == 0:
                        inflight[(p, h, nh)] = pgen.tile(
                            [P, 512], F32, tag="pv", name=f"pv{p}_{h}_{nh}")
                    pvt = inflight[(p, h, nh)]
                    for mt in range(4 * half, 4 * half + 4):
                        et = et_tiles[(p, mt)]
                        nc.tensor.matmul(
                            pvt[0:VW, :],
                            lhsT=v_sb[mt][:, hg * VW:(hg + 1) * VW],
                            rhs=et[:, nh * N_TOK + h * 512:
                                   nh * N_TOK + (h + 1) * 512],
                            start=(mt == 0), stop=(mt == NP_T - 1),
                        )
                    if half == 1:
                        if nh == 0:
                            inflight[("stg", p, h)] = stgp.tile(
                                [VW, N_TOK], F32, tag="stg", name=f"stg{hg}")
                        stg = inflight[("stg", p, h)]
                        if p == NPAIRS - 1:
                            # ScalarE is idle by now; keep the DVE free
                            nc.scalar.copy(
                                out=stg[:, nh * 512:(nh + 1) * 512],
                                in_=pvt[0:VW, :])
                        else:
                            nc.vector.tensor_copy(
                                out=stg[:, nh * 512:(nh + 1) * 512],
                                in_=pvt[0:VW, :])
                        del inflight[(p, h, nh)]
                        if nh == 1:
                            normalize_evict(p, h, stg)
                            del inflight[("stg", p, h)]
                            done = inflight.setdefault(("norm", p), set())
                            done.add(h)
                            if len(done) == 2:
                                for mt in range(NP_T):
                                    del et_tiles[(p, mt)]

                FILLER_KS = {0: (0, 1, 2), 1: (3, 4, 5), 2: (6, 7)}

                def filler_chunk(pnext, mt, wq_next, wk_next):
                    """QKT matmuls for pair pnext: M-tile qt (slots 0-2) or
                    kt (slots 3-5), k-values grouped 3/3/2 so each tile's
                    eviction lands a slot early (the next pair's first S^T
                    must not wait on the kt CAST). Returns the SBUF tile
                    after the last chunk."""
                    if mt > 5:
                        return None
                    is_kt = mt >= 3
                    j = (DP + pnext) if is_kt else pnext
                    wblk = wk_next if is_kt else wq_next
                    s = mt % 3
                    key = ("fill", pnext, is_kt)
                    if s == 0:
                        inflight[key] = pq2.tile([P, N_TOK], F32, tag="pq",
                                                 name=f"psf{j}")
                    ps = inflight[key]
                    for k in FILLER_KS[s]:
                        for nh in range(2):
                            nc.tensor.matmul(
                                ps[:, nh * 512:(nh + 1) * 512],
                                lhsT=wblk[:, k * P:(k + 1) * P],
                                rhs=xs(k)[:, nh * 512:(nh + 1) * 512],
                                start=(k == 0), stop=(k == DP - 1),
                            )
                    if s == 2:
                        t = qktp.tile([P, N_TOK], _DT, tag="qkt",
                                      name=f"qkt{j}")
                        nc.vector.tensor_copy(out=t[:], in_=ps[:])
                        del inflight[key]
                        return t
                    return None

                def emit_st_exp(p, mt):
                    """S^T + exp for (p, mt): two [128, 1024] psum chunks
                    (nh-major), each exp'd separately so the next slot's
                    S^T only waits on the matching chunk's exp."""
                    et = etp.tile([P, 2 * N_TOK], _DT, tag="et",
                                  name=f"et{p}_{mt}")
                    for nh in range(2):
                        st = pstp.tile([P, N_TOK], F32, tag="pst",
                                       name=f"st{p}_{mt}_{nh}")
                        for h in range(2):
                            nc.tensor.matmul(
                                st[:, h * 512:(h + 1) * 512],
                                lhsT=kt_cur[h * D_K:(h + 1) * D_K,
                                            mt * P:(mt + 1) * P],
                                rhs=qt_cur[h * D_K:(h + 1) * D_K,
                                           nh * 512:(nh + 1) * 512],
                                start=True, stop=True,
                                tile_position=(h * D_K, 0),
                            )
                        nc.scalar.activation(
                            et[:, nh * N_TOK:(nh + 1) * N_TOK], st[:],
                            mybir.ActivationFunctionType.Exp,
                            scale=float(SCALE))
                    et_tiles[(p, mt)] = et

                # projection: wave A = pairs 0-4 (pair-7 loop), wave B1 =
                # pairs 5-6 (early drain, += oacc), wave B2 = pair 7.
                oacc_all = wvoa.tile([P, DP * DIM], _DT, tag="wvoa",
                                     name="oacc")

                def oaccs(j):
                    return oacc_all[:, j * DIM:(j + 1) * DIM]

                def proj_wave_a(j):
                    ps = pq2.tile([P, DIM], F32, tag="pq", name=f"pso{j}")
                    for p in range(5):
                        for nh in range(2):
                            nc.tensor.matmul(
                                ps[:, nh * 512:(nh + 1) * 512],
                                lhsT=attnT_sb[p][:, j * P:(j + 1) * P],
                                rhs=wouts(p)[:, nh * 512:(nh + 1) * 512],
                                start=(p == 0), stop=(p == 4),
                            )
                    nc.vector.tensor_add(out=oaccs(j), in0=ps[:],
                                         in1=bias_bc[:])

                # ---- the pair loop ----
                for p in range(NPAIRS):
                    qt_next = kt_next = None
                    if p + 1 < NPAIRS:
                        wq_next = wqkb.tile([P, DIM], _DT, tag="wq",
                                            name=f"wq{p + 1}")
                        nc.sync.dma_start(wq_next[:], wq_block(p + 1))
                        wk_next = wqkb.tile([P, DIM], _DT, tag="wk",
                                            name=f"wk{p + 1}")
                        nc.sync.dma_start(wk_next[:], wq_block(DP + p + 1))
                    for mt in range(NP_T):
                        if p + 1 < NPAIRS:
                            t = filler_chunk(p + 1, mt, wq_next, wk_next)
                            if t is not None:
                                if mt < 3:
                                    qt_next = t
                                else:
                                    kt_next = t
                        emit_st_exp(p, mt)
                        if p == 0:
                            emit_v_chunk(mt, 0)
                            emit_v_chunk(mt, 1)
                        if p > 0:
                            pv_chunk(p - 1, mt)
                        if p == NPAIRS - 1:
                            proj_wave_a(mt)
                    qt_cur, kt_cur = qt_next, kt_next

                # ==== drain: last pair's PV interleaved with projection
                # wave B (pairs 5-7 per token tile, one evict add each).
                # Three psum groups rotate (pstp bufs=2 + pq2) so a late
                # attnT_7 never serializes the j sweep. ====
                with tc.tile_pool(name="ev", bufs=2) as ev:
                    bps = {}

                    def wave_b_p56(j):
                        pool = pq2 if j % 3 == 2 else pstp
                        tag = "pq" if j % 3 == 2 else "pst"
                        ps = bps[j] = pool.tile([P, DIM], F32, tag=tag,
                                                name=f"psb{j}")
                        for p in (5, 6):
                            for nh in range(2):
                                nc.tensor.matmul(
                                    ps[:, nh * 512:(nh + 1) * 512],
                                    lhsT=attnT_sb[p][:, j * P:(j + 1) * P],
                                    rhs=wouts(p)[:, nh * 512:(nh + 1) * 512],
                                    start=(p == 5), stop=False,
                                )

                    def wave_b_p7(j):
                        ps = bps.pop(j)
                        p = NPAIRS - 1
                        for nh in range(2):
                            nc.tensor.matmul(
                                ps[:, nh * 512:(nh + 1) * 512],
                                lhsT=attnT_sb[p][:, j * P:(j + 1) * P],
                                rhs=wouts(p)[:, nh * 512:(nh + 1) * 512],
                                start=False, stop=True,
                            )
                        o = ev.tile([P, DIM], F32, tag="out", name=f"o{j}")
                        nc.vector.tensor_add(out=o[:], in0=ps[:],
                                             in1=oaccs(j))
                        dma = nc.sync.dma_start if j % 2 else \
                            nc.gpsimd.dma_start
                        dma(out[j * P:(j + 1) * P, :], o[:])

                    for slot8 in (4, 5, 6, 7, 0, 1, 2, 3):
                        pv_chunk(NPAIRS - 1, slot8)
                    wave_b_p56(0)
                    wave_b_p56(1)
                    wave_b_p56(2)
                    for j in range(NP_T):
                        wave_b_p7(j)
                        if j + 3 < NP_T:
                            wave_b_p56(j + 3)

    nc.compile()
    return nc


_NC_CACHE = None


def _get_program():
    global _NC_CACHE
    if _NC_CACHE is None:
        _NC_CACHE = build_program()
    return _NC_CACHE


def make_in_maps(x, w_qkv, w_out, b_out):
    w_qkv_c = np.ascontiguousarray(w_qkv).astype(_NPDT)
    w_out_c = np.ascontiguousarray(w_out).astype(_NPDT)
    b_out_c = np.ascontiguousarray(b_out).astype(np.float32)
    in_maps = []
    for b in range(N_CORES):
        xTb = np.ascontiguousarray(np.asarray(x[b]).T).astype(_NPDT)
        in_maps.append({
            "xT": xTb,
            "w_qkv": w_qkv_c,
            "w_out": w_out_c,
            "b_out": b_out_c,
        })
    return in_maps


def kernel(x, w_qkv, w_out, b_out):
    nc = _get_program()
    in_maps = make_in_maps(x, w_qkv, w_out, b_out)
    res = run_bass_kernel_spmd(nc, in_maps, list(range(N_CORES)))
    outs = [np.asarray(r["out"], dtype=np.float32) for r in res.results]
    return np.stack(outs, axis=0)



# revision 1
# speedup vs baseline: 1.1745x; 1.1745x over previous
"""ViT attention block (B=8, N=1024, dim=1024, heads=16, d_k=64) on 8 trn2 NeuronCores.

Sharding: data-parallel over batch (1 batch per core), weights replicated.
No collectives needed; each core computes its batch's full attention output.

Per-core algorithm (all matmuls on TensorE contract over the partition dim):
  - host pre-transposes x[b] -> xT [dim, tokens] so QKV projections can use
    w_qkv (natural layout) as the stationary operand.
  - QT/KT = (w_qkv[:, :2048]).T @ xT  -> [2048, tokens]; head pair 2t,2t+1
    lives in partition-tile t ([128, 1024]), i.e. heads' d_k=64 rows stacked.
  - V = xT.T @ w_qkv[:, 2048:]       -> [tokens, 1024], stored with a
    constant-1 column appended per head (65 cols/head) so the PV matmul
    produces softmax row-sums for free.
  - per head pair: S^T[m,n] = (KT tile).T @ QT (K=64 contraction; the two
    heads run as concurrent row-group matmuls via tile_position). S^T psum
    is chunked into two [128, 1024] tiles (2 banks each, double-buffered)
    so the exp of chunk i never blocks the S^T matmuls of chunk i+1 on a
    PSUM bank. exp(scale*S^T) on ScalarE streams per-chunk into ET bf16,
    laid out [128, (nh, h, 512)] so both exp chunks and PV rhs slices are
    contiguous. (max-subtraction is skipped: |scale*S| <~ 2 here, exp is
    exact-safe and softmax is shift-invariant.)
  - PV: out^T[d'+1, n] = V'.T @ E^T accumulated over m tiles; row 64 is the
    softmax denominator. Stage to SBUF, then: SBUF->SBUF reshape DMA of the
    denominator row to [128, 8] (reciprocal is ~12 cyc/elem on DVE, so it
    must run on the wide-partition layout), reciprocal, bounce through a
    DRAM row, and a stride-0-partition broadcast read back to [64, 1024].
    The normalize multiplies run on GpSimd (SBUF-only operands) to keep
    the DVE free for psum evictions.
  - final = attnT.T @ w_out + b_out, accumulated in three waves so no
    projection matmul ever waits on a late normalize chain: wave A
    (pairs 0-4, inside pair-7's loop, bias folded) -> oacc; wave B1
    (pairs 5-6, early drain, += oacc in place); wave B2 (pair 7, late
    drain) -> evict fp32 + out DMA, alternating sync/gpsimd DMA queues
    to double drain write bandwidth.

Startup: inputs land via 7 large DMA descriptors (the sync queue issues
descriptors at ~600 ns each, so many small DMAs would serialize the
start): xT in 2, pair-0 q/k column blocks via strided 3D APs, wv, wout,
bias. Loop fillers stream the remaining q/k blocks one pair ahead.
"""

import os
import numpy as np
import ml_dtypes

import concourse.bass as bass
from concourse import bacc
import concourse.mybir as mybir
import concourse.tile as tile
from concourse.bass_utils import run_bass_kernel_spmd

P = 128
N_TOK = 1024
DIM = 1024
HEADS = 16
D_K = 64
N_CORES = 8
SCALE = D_K ** -0.5  # 0.125

NP_T = N_TOK // P   # 8 token tiles
DP = DIM // P       # 8 dim tiles
NPAIRS = HEADS // 2  # 8 head pairs
VW = D_K + 1        # 65: V columns per head incl. ones column

# matmul operand dtype: "bf16" | "fp32r" | "fp32"
MM_DTYPE = os.environ.get("KERNEL_MM_DTYPE", "bf16")
_DT = {
    "bf16": mybir.dt.bfloat16,
    "fp32r": mybir.dt.float32r,
    "fp32": mybir.dt.float32,
}[MM_DTYPE]
_NPDT = {"bf16": ml_dtypes.bfloat16, "fp32r": np.float32, "fp32": np.float32}[MM_DTYPE]

F32 = mybir.dt.float32


def build_program():
    nc = bacc.Bacc("TRN2", target_bir_lowering=False, debug=False)

    xT = nc.dram_tensor("xT", [DIM, N_TOK], _DT, kind="ExternalInput").ap()
    wqkv = nc.dram_tensor("w_qkv", [DIM, 3 * DIM], _DT, kind="ExternalInput").ap()
    wout = nc.dram_tensor("w_out", [DIM, DIM], _DT, kind="ExternalInput").ap()
    bout = nc.dram_tensor("b_out", [DIM], F32, kind="ExternalInput").ap()
    out = nc.dram_tensor("out", [N_TOK, DIM], F32, kind="ExternalOutput").ap()
    # reciprocal'd softmax denominator bounce rows (one per head)
    rs_dram = nc.dram_tensor("rs_scratch", [HEADS, N_TOK], F32).ap()

    def wq_block(j):
        """One DMA-able view of w_qkv column block j (128 cols) across all
        1024 rows: [p=128, k=8, c=128] -> lands as an SBUF [128, 1024] tile
        whose k-th 128-col slice is w_qkv[k*128:(k+1)*128, j*128:(j+1)*128]."""
        return bass.AP(
            tensor=wqkv.tensor, offset=wqkv.offset + j * P,
            ap=[[3 * DIM, P], [P * 3 * DIM, DP], [1, P]],
        )

    with tile.TileContext(nc) as tc:
        with (
            tc.tile_pool(name="persist", bufs=1) as persist,
            tc.tile_pool(name="qkt", bufs=5) as qktp,
            tc.tile_pool(name="wqkb", bufs=2) as wqkb,
            tc.tile_pool(name="wvoa", bufs=1) as wvoa,
            tc.tile_pool(name="et", bufs=18) as etp,
            tc.tile_pool(name="stg", bufs=3) as stgp,
            tc.tile_pool(name="small", bufs=2) as small,
        ):
            v_sb = []      # per token-tile: [128, 16*65]
            attnT_sb = []  # per pair: [128, 1024] = two heads' [64, n]
            for j in range(NP_T):
                v_sb.append(persist.tile([P, HEADS * VW], _DT, tag=f"v{j}",
                                         name=f"v{j}"))
            for p in range(NPAIRS):
                attnT_sb.append(persist.tile([P, N_TOK], _DT, tag=f"attnT{p}",
                                             name=f"attnT{p}"))

            # ---- input DMAs spread across 4 queues: each queue issues
            # descriptors at ~600 ns, so parallel queues beat one long
            # chain. sync: xT (arrival-paced thirds); vector: pair-0 wq
            # per-k tiles; scalar: pair-0 wk per-k tiles; gpsimd: wv,
            # wout, bias. ----
            xT_all = persist.tile([P, DP * N_TOK], _DT, tag="xT", name="xT")
            for k in range(DP):
                nc.sync.dma_start(
                    xT_all[:, k * N_TOK:(k + 1) * N_TOK],
                    xT[k * P:(k + 1) * P, :])

            def xs(k):
                return xT_all[:, k * N_TOK:(k + 1) * N_TOK]

            wq_cur = wqkb.tile([P, DIM], _DT, tag="wq", name="wq0")
            wk_cur = wqkb.tile([P, DIM], _DT, tag="wk", name="wk0")
            for k in range(DP):
                nc.scalar.dma_start(wq_cur[:, k * P:(k + 1) * P],
                                    wqkv[k * P:(k + 1) * P, 0:P])
                nc.sync.dma_start(wk_cur[:, k * P:(k + 1) * P],
                                  wqkv[k * P:(k + 1) * P,
                                       DP * P:DP * P + P])

            wv_all = wvoa.tile([P, DP * DIM], _DT, tag="wvoa", name="wv")
            for k0 in range(0, DP, 2):
                src = bass.AP(
                    tensor=wqkv.tensor,
                    offset=wqkv.offset + k0 * P * 3 * DIM + 2 * DIM,
                    ap=[[3 * DIM, P], [P * 3 * DIM, 2], [1, DIM]])
                nc.gpsimd.dma_start(
                    wv_all[:, k0 * DIM:(k0 + 2) * DIM], src)

            def wvs(k):
                return wv_all[:, k * DIM:(k + 1) * DIM]

            wout_all = persist.tile([P, DP * DIM], _DT, tag="wout",
                                    name="wout")
            for k0 in range(0, DP, 2):
                src = bass.AP(
                    tensor=wout.tensor, offset=wout.offset + k0 * P * DIM,
                    ap=[[DIM, P], [P * DIM, 2], [1, DIM]])
                nc.gpsimd.dma_start(
                    wout_all[:, k0 * DIM:(k0 + 2) * DIM], src)

            def wouts(k):
                return wout_all[:, k * DIM:(k + 1) * DIM]

            bias_bc = persist.tile([P, DIM], F32, tag="bias")
            bias_in = bass.AP(tensor=bout.tensor, offset=bout.offset,
                              ap=[[0, P]] + list(bout.ap))
            nc.gpsimd.dma_start(bias_bc[:], bias_in)
            for j in range(NP_T):
                nc.vector.memset(
                    v_sb[j][:].rearrange("p (h x) -> p h x", x=VW)[:, :, D_K:],
                    1.0)

            def emit_qkt(j, wblk, pool):
                """QKT M-tile j ([128, tokens] slice of QKV^T) in full."""
                ps = pool.tile([P, N_TOK], F32, tag="pq", name=f"psqk{j}")
                for k in range(DP):
                    for nh in range(2):
                        nc.tensor.matmul(
                            ps[:, nh * 512:(nh + 1) * 512],
                            lhsT=wblk[:, k * P:(k + 1) * P],
                            rhs=xs(k)[:, nh * 512:(nh + 1) * 512],
                            start=(k == 0), stop=(k == DP - 1),
                        )
                t = qktp.tile([P, N_TOK], _DT, tag="qkt", name=f"qkt{j}")
                nc.vector.tensor_copy(out=t[:], in_=ps[:])
                return t

            # ============ phase 1: pair-0 QT/KT, k-interleaved so both
            # psums accumulate as the xT / weight DMAs arrive ============
            with tc.tile_pool(name="pq1", bufs=2, space="PSUM") as pq1:
                psq = pq1.tile([P, N_TOK], F32, tag="pq", name="psqk0")
                psk = pq1.tile([P, N_TOK], F32, tag="pq", name="psqk8")
                for k in range(DP):
                    for ps, wblk in ((psq, wq_cur), (psk, wk_cur)):
                        for nh in range(2):
                            nc.tensor.matmul(
                                ps[:, nh * 512:(nh + 1) * 512],
                                lhsT=wblk[:, k * P:(k + 1) * P],
                                rhs=xs(k)[:, nh * 512:(nh + 1) * 512],
                                start=(k == 0), stop=(k == DP - 1),
                            )
                qt_cur = qktp.tile([P, N_TOK], _DT, tag="qkt", name="qkt0")
                nc.vector.tensor_copy(out=qt_cur[:], in_=psq[:])
                kt_cur = qktp.tile([P, N_TOK], _DT, tag="qkt", name="qkt8")
                nc.vector.tensor_copy(out=kt_cur[:], in_=psk[:])

            # ============ phase 2: pipelined attention ============
            with (
                tc.tile_pool(name="pst", bufs=2, space="PSUM") as pstp,
                tc.tile_pool(name="pq2", bufs=1, space="PSUM") as pq2,
                tc.tile_pool(name="pgen", bufs=2, space="PSUM") as pgen,
            ):
                et_tiles = {}   # (pair, mt) -> ET tile [128, 2048]
                inflight = {}   # accumulation state for pv / filler psums

                def emit_v_chunk(j, nh):
                    """8 matmuls: V[j-tile, nh-half] = xT.T @ wv, evicted
                    into v_sb[j] (heads nh*8..nh*8+7, 64 cols each)."""
                    ps = pgen.tile([P, 512], F32, tag="pv",
                                   name=f"psv{j}_{nh}")
                    for k in range(DP):
                        nc.tensor.matmul(
                            ps[:],
                            lhsT=xs(k)[:, j * P:(j + 1) * P],
                            rhs=wvs(k)[:, nh * 512:(nh + 1) * 512],
                            start=(k == 0), stop=(k == DP - 1),
                        )
                    nc.vector.tensor_copy(
                        out=v_sb[j][:].rearrange(
                            "p (h x) -> p h x", x=VW)[:, 8 * nh:8 * nh + 8,
                                                      :D_K],
                        in_=ps[:].rearrange("p (h d) -> p h d", d=D_K),
                    )

                def normalize_evict(p, h, stg):
                    """Normalize the staged PV result by the softmax
                    denominator (row 64) and write into attnT_sb[p].
                    Latency chains ride the gpsimd SWDGE queue to keep the
                    sync queue free for weight streams; the last pair uses
                    the (by then idle) sync queue for lower latency."""
                    hg = 2 * p + h
                    dma = nc.gpsimd.dma_start if p < NPAIRS - 1 else \
                        nc.sync.dma_start
                    rsp = small.tile([P, NP_T], F32, tag="rsp",
                                     name=f"rsp{hg}")
                    dma(rsp[:], stg[D_K:VW, :].rearrange(
                        "o (p i) -> o p i", p=P))
                    rsq = small.tile([P, NP_T], F32, tag="rsq",
                                     name=f"rsq{hg}")
                    nc.vector.reciprocal(rsq[:], rsp[:])
                    dma(rs_dram[hg].rearrange("(p i) -> p i", p=P), rsq[:])
                    rs_row = rs_dram[hg:hg + 1, :]
                    rs_bc = bass.AP(tensor=rs_row.tensor, offset=rs_row.offset,
                                    ap=[[0, D_K], list(rs_row.ap)[-1]])
                    rrec = small.tile([D_K, N_TOK], F32, tag="rrec",
                                      name=f"rrec{hg}")
                    dma(rrec[:], rs_bc)
                    if h == 0:
                        nc.vector.tensor_mul(out=attnT_sb[p][0:D_K, :],
                                             in0=stg[0:D_K, :], in1=rrec[:])
                    else:
                        tmp = small.tile([D_K, N_TOK], _DT, tag="oddtmp",
                                         name=f"oddtmp{hg}")
                        nc.vector.tensor_mul(out=tmp[:],
                                             in0=stg[0:D_K, :], in1=rrec[:])
                        dma(attnT_sb[p][D_K:P, :], tmp[:])

                def pv_chunk(p, slot8):
                    """4 PV matmuls for pair p. Passes of 8 MMs: (h, nh) =
                    slot8//2, each pass covers all m-tiles in 2 slots using
                    a 1-bank psum tile; evicted into the stg half."""
                    h, nh = slot8 // 4, (slot8 // 2) % 2
                    hg = 2 * p + h
                    half = slot8 % 2  # first or second 4 m-tiles
                    if half == 0:
                        inflight[(p, h, nh)] = pgen.tile(
                            [P, 512], F32, tag="pv", name=f"pv{p}_{h}_{nh}")
                    pvt = inflight[(p, h, nh)]
                    for mt in range(4 * half, 4 * half + 4):
                        et = et_tiles[(p, mt)]
                        nc.tensor.matmul(
                            pvt[0:VW, :],
                            lhsT=v_sb[mt][:, hg * VW:(hg + 1) * VW],
                            rhs=et[:, nh * N_TOK + h * 512:
                                   nh * N_TOK + (h + 1) * 512],
                            start=(mt == 0), stop=(mt == NP_T - 1),
                        )
                    if half == 1:
                        if nh == 0:
                            inflight[("stg", p, h)] = stgp.tile(
                                [VW, N_TOK], F32, tag="stg", name=f"stg{hg}")
                        stg = inflight[("stg", p, h)]
                        if p == NPAIRS - 1:
                            # ScalarE is idle by now; keep the DVE free
                            nc.scalar.copy(
                                out=stg[:, nh * 512:(nh + 1) * 512],
                                in_=pvt[0:VW, :])
                        else:
                            nc.vector.tensor_copy(
                                out=stg[:, nh * 512:(nh + 1) * 512],
                                in_=pvt[0:VW, :])
                        del inflight[(p, h, nh)]
                        if nh == 1:
                            normalize_evict(p, h, stg)
                            del inflight[("stg", p, h)]
                            done = inflight.setdefault(("norm", p), set())
                            done.add(h)
                            if len(done) == 2:
                                for mt in range(NP_T):
                                    del et_tiles[(p, mt)]

                FILLER_KS = {0: (0, 1, 2), 1: (3, 4, 5), 2: (6, 7)}

                def filler_chunk(pnext, mt, wq_next, wk_next):
                    """QKT matmuls for pair pnext: M-tile qt (slots 0-2) or
                    kt (slots 3-5), k-values grouped 3/3/2 so each tile's
                    eviction lands a slot early (the next pair's first S^T
                    must not wait on the kt CAST). Returns the SBUF tile
                    after the last chunk."""
                    if mt > 5:
                        return None
                    is_kt = mt >= 3
                    j = (DP + pnext) if is_kt else pnext
                    wblk = wk_next if is_kt else wq_next
                    s = mt % 3
                    key = ("fill", pnext, is_kt)
                    if s == 0:
                        inflight[key] = pq2.tile([P, N_TOK], F32, tag="pq",
                                                 name=f"psf{j}")
                    ps = inflight[key]
                    for k in FILLER_KS[s]:
                        for nh in range(2):
                            nc.tensor.matmul(
                                ps[:, nh * 512:(nh + 1) * 512],
                                lhsT=wblk[:, k * P:(k + 1) * P],
                                rhs=xs(k)[:, nh * 512:(nh + 1) * 512],
                                start=(k == 0), stop=(k == DP - 1),
                            )
                    if s == 2:
                        t = qktp.tile([P, N_TOK], _DT, tag="qkt",
                                      name=f"qkt{j}")
                        nc.vector.tensor_copy(out=t[:], in_=ps[:])
                        del inflight[key]
                        return t
                    return None

                def emit_st_exp(p, mt):
                    """S^T + exp for (p, mt): two [128, 1024] psum chunks
                    (nh-major), each exp'd separately so the next slot's
                    S^T only waits on the matching chunk's exp."""
                    et = etp.tile([P, 2 * N_TOK], _DT, tag="et",
                                  name=f"et{p}_{mt}")
                    for nh in range(2):
                        st = pstp.tile([P, N_TOK], F32, tag="pst",
                                       name=f"st{p}_{mt}_{nh}")
                        for h in range(2):
                            nc.tensor.matmul(
                                st[:, h * 512:(h + 1) * 512],
                                lhsT=kt_cur[h * D_K:(h + 1) * D_K,
                                            mt * P:(mt + 1) * P],
                                rhs=qt_cur[h * D_K:(h + 1) * D_K,
                                           nh * 512:(nh + 1) * 512],
                                start=True, stop=True,
                                tile_position=(h * D_K, 0),
                            )
                        nc.scalar.activation(
                            et[:, nh * N_TOK:(nh + 1) * N_TOK], st[:],
                            mybir.ActivationFunctionType.Exp,
                            scale=float(SCALE))
                    et_tiles[(p, mt)] = et

                # projection: wave A = pairs 0-4 (pair-7 loop), wave B1 =
                # pairs 5-6 (early drain, += oacc), wave B2 = pair 7.
                oacc_all = wvoa.tile([P, DP * DIM], _DT, tag="wvoa",
                                     name="oacc")

                def oaccs(j):
                    return oacc_all[:, j * DIM:(j + 1) * DIM]

                def proj_wave_a(j):
                    ps = pq2.tile([P, DIM], F32, tag="pq", name=f"pso{j}")
                    for p in range(5):
                        for nh in range(2):
                            nc.tensor.matmul(
                                ps[:, nh * 512:(nh + 1) * 512],
                                lhsT=attnT_sb[p][:, j * P:(j + 1) * P],
                                rhs=wouts(p)[:, nh * 512:(nh + 1) * 512],
                                start=(p == 0), stop=(p == 4),
                            )
                    nc.vector.tensor_add(out=oaccs(j), in0=ps[:],
                                         in1=bias_bc[:])

                # ---- the pair loop ----
                for p in range(NPAIRS):
                    qt_next = kt_next = None
                    if p + 1 < NPAIRS:
                        wq_next = wqkb.tile([P, DIM], _DT, tag="wq",
                                            name=f"wq{p + 1}")
                        nc.sync.dma_start(wq_next[:], wq_block(p + 1))
                        wk_next = wqkb.tile([P, DIM], _DT, tag="wk",
                                            name=f"wk{p + 1}")
                        nc.sync.dma_start(wk_next[:], wq_block(DP + p + 1))
                    for mt in range(NP_T):
                        if p + 1 < NPAIRS:
                            t = filler_chunk(p + 1, mt, wq_next, wk_next)
                            if t is not None:
                                if mt < 3:
                                    qt_next = t
                                else:
                                    kt_next = t
                        emit_st_exp(p, mt)
                        if p == 0:
                            emit_v_chunk(mt, 0)
                            emit_v_chunk(mt, 1)
                        if p > 0:
                            pv_chunk(p - 1, mt)
                        if p == NPAIRS - 1:
                            proj_wave_a(mt)
                    qt_cur, kt_cur = qt_next, kt_next

                # ==== drain: last pair's PV interleaved with projection
                # wave B (pairs 5-7 per token tile, one evict add each).
                # Three psum groups rotate (pstp bufs=2 + pq2) so a late
                # attnT_7 never serializes the j sweep. ====
                with tc.tile_pool(name="ev", bufs=2) as ev:
                    bps = {}

                    def wave_b_p56(j):
                        pool = pq2 if j % 3 == 2 else pstp
                        tag = "pq" if j % 3 == 2 else "pst"
                        ps = bps[j] = pool.tile([P, DIM], F32, tag=tag,
                                                name=f"psb{j}")
                        for p in (5, 6):
                            for nh in range(2):
                                nc.tensor.matmul(
                                    ps[:, nh * 512:(nh + 1) * 512],
                                    lhsT=attnT_sb[p][:, j * P:(j + 1) * P],
                                    rhs=wouts(p)[:, nh * 512:(nh + 1) * 512],
                                    start=(p == 5), stop=False,
                                )

                    def wave_b_p7(j):
                        ps = bps.pop(j)
                        p = NPAIRS - 1
                        for nh in range(2):
                            nc.tensor.matmul(
                                ps[:, nh * 512:(nh + 1) * 512],
                                lhsT=attnT_sb[p][:, j * P:(j + 1) * P],
                                rhs=wouts(p)[:, nh * 512:(nh + 1) * 512],
                                start=False, stop=True,
                            )
                        o = ev.tile([P, DIM], F32, tag="out", name=f"o{j}")
                        nc.vector.tensor_add(out=o[:], in0=ps[:],
                                             in1=oaccs(j))
                        dma = nc.sync.dma_start if j % 2 else \
                            nc.gpsimd.dma_start
                        dma(out[j * P:(j + 1) * P, :], o[:])

                    for slot8 in (4, 5, 6, 7, 0, 1, 2, 3):
                        pv_chunk(NPAIRS - 1, slot8)
                    wave_b_p56(0)
                    wave_b_p56(1)
                    wave_b_p56(2)
                    for j in range(NP_T):
                        wave_b_p7(j)
                        if j + 3 < NP_T:
                            wave_b_p56(j + 3)

    nc.compile()
    return nc


_NC_CACHE = None


def _get_program():
    global _NC_CACHE
    if _NC_CACHE is None:
        _NC_CACHE = build_program()
    return _NC_CACHE


def make_in_maps(x, w_qkv, w_out, b_out):
    w_qkv_c = np.ascontiguousarray(w_qkv).astype(_NPDT)
    w_out_c = np.ascontiguousarray(w_out).astype(_NPDT)
    b_out_c = np.ascontiguousarray(b_out).astype(np.float32)
    in_maps = []
    for b in range(N_CORES):
        xTb = np.ascontiguousarray(np.asarray(x[b]).T).astype(_NPDT)
        in_maps.append({
            "xT": xTb,
            "w_qkv": w_qkv_c,
            "w_out": w_out_c,
            "b_out": b_out_c,
        })
    return in_maps


def kernel(x, w_qkv, w_out, b_out):
    nc = _get_program()
    in_maps = make_in_maps(x, w_qkv, w_out, b_out)
    res = run_bass_kernel_spmd(nc, in_maps, list(range(N_CORES)))
    outs = [np.asarray(r["out"], dtype=np.float32) for r in res.results]
    return np.stack(outs, axis=0)

